# revision 30
# baseline (speedup 1.0000x reference)
"""DeepInterestNetwork (DIN) forward — Trainium2 Bass kernel, 8-core SPMD.

Distribution: pure data-parallel over the batch (4096 -> 512 per core).
The full embedding table (flattened [20*200001, 64]) is passed to every
core and gathered on-device via indirect DMA ([128,1]-offset form — the
only shape the HW descriptor generator honors).

Key optimizations over the v1 kernel:
  - batch rows are sorted per-core by history_length (descending) so
    history chunks whose whole 128-row block is masked (l >= max hlen in
    block) are never gathered — hraw/histT2 are pre-zeroed instead and
    the softmax mask makes the math exact. Cuts ~25 of 76 hist gathers;
    each [128,1] indirect gather costs ~1.1us of fixed SWDGE overhead on
    the gpsimd engine, which is the critical path.
  - fp32r matmuls (1 cyc/row at N=512 vs 4 for fp32) and fp32r PE
    transposes (1.5 vs 2 cyc/row).
  - final-DNN K-chunk accumulation interleaved with the emb transposes
    so the tail after the last gather is only the last 2 chunks + MLP.

Per-core dataflow (activations feature-major "transposed" so PE contracts
over features; gather chunk order is chosen so every PE transpose reads a
contiguous [128, 128] block):
  gather emb rows (f-pair-major) + hist rows (l-pair-major, pruned)
  PE-transpose pairs -> XT slots / histT2 / quT2 / f19T
  attention MLP (3 matmul stages, l-pairs packed on partition halves)
  softmax over history (batch-partitioned [128, 20] psum tiles)
  weighted pooling (DVE mul+reduce in gather layout) -> poolT
  final DNN (K-chunked accumulating matmuls) -> y [1, 512]
"""

import numpy as np

import concourse.bass as bass
import concourse.bacc as bacc
import concourse.tile as tile
from concourse import mybir
from concourse.bass_utils import run_bass_kernel_spmd

f32 = mybir.dt.float32
f32r = mybir.dt.float32r
i32 = mybir.dt.int32
AF = mybir.ActivationFunctionType
ALU = mybir.AluOpType

# ---- problem sizes (hardcoded per the harness contract) ----
NCORES = 8
B = 4096
BC = B // NCORES          # 512 batch rows per core
NB = BC // 128            # 4 batch chunks of 128
NF = 20
V1 = 200001
D = 64
L = 19
L2 = 20                   # history padded to even count
G = L2 // 2               # 10 l-pairs
EC = NF * NB              # 80 emb gather chunks of 128 rows
NEG_BIG = -1.0e30

# fp32r: same fp32 bits, 1 cyc/row PE rate at N>=256 (vs 4 for fp32).
USE_FP32R = True

# ---- gather chunk orders (shared by host index prep and device AP math) --
# emb chunks: f0 singles, f19 singles (both early: short consumer chains),
# then (f=2k-1 / f=2k) pairs per b-block
ECHUNKS = (
    [(0, bb) for bb in range(NB)]
    + [(19, bb) for bb in range(NB)]
    + [
        (2 * k - 1 + par, bb)
        for k in range(1, G)
        for bb in range(NB)
        for par in range(2)
    ]
)
# hist chunk grid position for (l, bb): pairs (l=2g, l=2g+1) adjacent
def _hpos(l, bb):
    return (l // 2) * (2 * NB) + bb * 2 + (l % 2)


def hchunks_for_plan(maxlen):
    """Gathered history chunks given per-block max hlen (batch sorted desc)."""
    return [
        (l, bb) for l in range(L) for bb in range(NB) if l < maxlen[bb]
    ]


def _mm(ap):
    return ap.bitcast(f32r) if USE_FP32R else ap


def _ap3(base_ap, dims):
    return bass.AP(base_ap.tensor, base_ap.offset, dims)


def build_program(maxlen):
    hchunks = hchunks_for_plan(maxlen)
    nh = len(hchunks)

    nc = bacc.Bacc(trn_type="TRN2")

    dram = {}

    def din(name, shape, dt=f32):
        dram[name] = nc.dram_tensor(name, shape, dt, kind="ExternalInput")
        return dram[name]

    din("table", [NF * V1, D])
    din("eidx", [128, EC], i32)
    din("hidx", [128, max(nh, 1)], i32)
    din("denseT", [D, BC])
    din("lidx", [128, L2])
    din("lenf", [128, NB])
    din("ident", [128, 128])
    din("aw1q2", [128, D])
    din("aw1h2", [128, D])
    din("ab1_2", [128, 1])
    din("aw2bd", [128, 32])
    din("ab2_8", [128, 1])
    din("aw3p", [128, 8])
    din("dw1p", [128, 12, 128])
    din("db1", [128, 1])
    din("dw2", [128, D])
    din("db2", [D, 1])
    din("dw3", [D, 1])
    din("db3", [1, 1])
    y_dram = nc.dram_tensor("y", [1, BC], f32, kind="ExternalOutput")

    with tile.TileContext(nc) as tc:
        with (
            tc.tile_pool(name="persist", bufs=1) as P,
            tc.tile_pool(name="work", bufs=3) as W,
            tc.tile_pool(name="pbig", bufs=3, space="PSUM") as PB,
            tc.tile_pool(name="pt", bufs=2, space="PSUM") as PT,
            tc.tile_pool(name="ps", bufs=1, space="PSUM") as PS,
            tc.tile_pool(name="pd", bufs=1, space="PSUM") as PD,
        ):
            # ---------------- input DMAs ----------------
            sb = {}

            def load(name, dt=f32, rnd=False):
                t = P.tile(list(dram[name].shape), dt, tag=name)
                if rnd:
                    # declare f32r so the BIR verifier accepts this tensor
                    # as an fp32r-matmul operand
                    nc.sync.dma_start(
                        out=t[:].bitcast(f32r), in_=dram[name][:].bitcast(f32r)
                    )
                else:
                    nc.sync.dma_start(out=t[:], in_=dram[name][:])
                sb[name] = t
                return t

            # split the eidx load so the first gathers (cols 0:8) start as
            # soon as a small DMA lands
            t_eidx = P.tile([128, EC], i32, tag="eidx")
            nc.sync.dma_start(out=t_eidx[:, 0:8], in_=dram["eidx"][:, 0:8])
            sb["eidx"] = t_eidx
            load("hidx", dt=i32)
            nc.sync.dma_start(out=t_eidx[:, 8:EC], in_=dram["eidx"][:, 8:EC])
            t_ident = load("ident")
            for w in ("aw1q2", "aw1h2", "dw1p", "dw2"):
                load(w, rnd=USE_FP32R)
            for w in (
                "aw2bd", "aw3p", "dw3",
                "ab1_2", "ab2_8", "db1", "db2", "db3", "lidx", "lenf",
            ):
                load(w)

            # XT slots 0..9 ([f0;dense], [f1;f2], ..., [f17;f18])
            t_XT = P.tile([128, G, BC], f32, tag="XT")
            nc.sync.dma_start(
                out=_mm(t_XT[64:128, 0, :]), in_=_mm(dram["denseT"][:])
            )
            t_f19T = P.tile([64, BC], f32, tag="f19T")
            t_poolT = P.tile([64, BC], f32, tag="poolT")

            t_negbig = P.tile([128, 1], f32, tag="negbig")
            nc.vector.memset(t_negbig[:], NEG_BIG)

            # ---------------- gathers ----------------
            t_eraw = P.tile([128, EC, D], f32, tag="eraw")
            t_hraw = P.tile([128, NB * L2, D], f32, tag="hraw")
            # zero only the non-gathered positions (masked chunks + l=19
            # pad) so the gathers carry no dependency on the memsets
            gathered = {_hpos(l, bb) for (l, bb) in hchunks}
            for pos in range(NB * L2):
                if pos not in gathered:
                    nc.vector.memset(t_hraw[:, pos, :], 0.0)
            # histT2 [128, G, 512]: partitions 0:64 = even l, 64:128 = odd l.
            # (masked chunks transpose zeros from the pre-zeroed hraw)
            t_histT2 = P.tile([128, G, BC], f32, tag="histT2")

            def gather(out_ap, idx_ap):
                return nc.gpsimd.indirect_dma_start(
                    out=out_ap,
                    out_offset=None,
                    in_=dram["table"][:],
                    in_offset=bass.IndirectOffsetOnAxis(ap=idx_ap, axis=0),
                )

            # f0 (query) + f19 first so their short consumer chains start
            # early, then hist, then the paired emb features
            for c in range(2 * NB):
                gather(t_eraw[:, c, :], sb["eidx"][:, c : c + 1])
            for ci, (l, bb) in enumerate(hchunks):
                gather(t_hraw[:, _hpos(l, bb), :], sb["hidx"][:, ci : ci + 1])
            for c in range(2 * NB, EC):
                gather(t_eraw[:, c, :], sb["eidx"][:, c : c + 1])

            cp_ctr = [0]

            def copy_alt(out_ap, in_ap):
                # out feeds fp32r matmuls: declare f32r so the copy rounds
                if cp_ctr[0] % 2 == 0:
                    nc.scalar.copy(out=_mm(out_ap), in_=in_ap)
                else:
                    nc.vector.tensor_copy(out=_mm(out_ap), in_=in_ap)
                cp_ctr[0] += 1

            def transpose(out_ap, lhsT_ap):
                nc.tensor.matmul(
                    out=out_ap, lhsT=lhsT_ap, rhs=t_ident[:],
                    is_transpose=True,
                )

            # final DNN layer-1 accumulation, interleaved chunk by chunk as
            # XT slots land. All target the same held psum tile pd1.
            pd1 = PD.tile([128, BC], f32, tag="pd")
            dnn_state = {"started": False}

            def dnn_chunk(k, lhsT_ap, rhs_ap, stop=False):
                nc.tensor.matmul(
                    out=pd1[:],
                    lhsT=_mm(lhsT_ap),
                    rhs=_mm(rhs_ap),
                    start=(not dnn_state["started"]),
                    stop=stop,
                    skip_group_check=True,
                )
                dnn_state["started"] = True

            # ---------- query transposes (f0 -> quT2 top + XT0 top) -------
            t_quT2 = P.tile([128, BC], f32, tag="quT2")
            for bb in range(NB):
                pq = PT.tile([128, 128], f32, tag="pt")
                transpose(pq[0:64, :], t_eraw[:, bb, :])
                cols = slice(bb * 128, (bb + 1) * 128)
                copy_alt(t_quT2[0:64, cols], pq[0:64, :])
                copy_alt(t_XT[0:64, 0, cols], pq[0:64, :])
            # replicate query d-rows onto partitions 64:127 (SBUF->SBUF DMA
            # is the only lane-crossing copy)
            nc.sync.dma_start(
                out=_mm(t_quT2[64:128, :]), in_=_mm(t_quT2[0:64, :])
            )
            # XT slot 0 complete (dense bottom half via DMA above)
            dnn_chunk(0, sb["dw1p"][:, 0, :], t_XT[:, 0, :])

            # f19 transposes + DNN chunk 10 (data gathered right after f0)
            for bb in range(NB):
                pf = PT.tile([128, 128], f32, tag="pt")
                transpose(pf[0:64, :], t_eraw[:, NB + bb, :])
                copy_alt(t_f19T[:, bb * 128 : (bb + 1) * 128], pf[0:64, :])
            dnn_chunk(10, sb["dw1p"][0:64, 10, :], t_f19T[:])

            # ---------------- hist transposes -> histT2 ----------------
            for g in range(G):
                for bb in range(NB):
                    c0 = g * (2 * NB) + bb * 2
                    ph = PT.tile([128, 128], f32, tag="pt")
                    transpose(ph[:], t_hraw[:, c0 : c0 + 2, :])
                    copy_alt(t_histT2[:, g, bb * 128 : (bb + 1) * 128], ph[:])

            # ---------------- emb transposes -> XT slots ----------------
            emb_pairs = [(k, bb) for k in range(1, G) for bb in range(NB)]

            def emit_emb_pair(k, bb):
                c0 = 2 * NB + (k - 1) * (2 * NB) + bb * 2
                pe_ = PT.tile([128, 128], f32, tag="pt")
                transpose(pe_[:], t_eraw[:, c0 : c0 + 2, :])
                copy_alt(t_XT[:, k, bb * 128 : (bb + 1) * 128], pe_[:])
                if bb == NB - 1:
                    dnn_chunk(k, sb["dw1p"][:, k, :], t_XT[:, k, :])

            # ---------------- attention mm1 + relu ----------------
            # two [64,·] psum tiles so both l-parity halves can be fp32r
            # (fp32r matmul dst must start at partition 0)
            t_h1T = P.tile([128, G, BC], f32, tag="h1T")
            for g in range(G):
                for h in range(2):
                    rows = slice(h * 64, (h + 1) * 64)
                    p1 = PB.tile([64, BC], f32, tag="pb")
                    nc.tensor.matmul(
                        out=p1[:],
                        lhsT=_mm(sb["aw1q2"][rows, :]),
                        rhs=_mm(t_quT2[rows, :]),
                        start=True, stop=False,
                    )
                    nc.tensor.matmul(
                        out=p1[:],
                        lhsT=_mm(sb["aw1h2"][rows, :]),
                        rhs=_mm(t_histT2[rows, g, :]),
                        start=False, stop=True,
                    )
                    nc.scalar.activation(
                        out=t_h1T[rows, g, :], in_=p1[:], func=AF.Relu,
                        bias=sb["ab1_2"][rows, :],
                    )

            # ---------------- attention mm2 + relu ----------------
            NT = (G + 3) // 4  # 3 psum tiles, quarters q = g%4
            t_h2T = P.tile([128, NT, BC], f32, tag="h2T")
            for t in range(NT):
                p2 = PB.tile([128, BC], f32, tag="pb")
                for q in range(min(4, G - 4 * t)):
                    g = 4 * t + q
                    rows = slice(q * 32, (q + 1) * 32)
                    nc.tensor.matmul(
                        out=p2[rows, :],
                        lhsT=sb["aw2bd"][:],
                        rhs=t_h1T[:, g, :],
                        start=True, stop=True, tile_position=(0, q * 32),
                    )
                nrows = 128 if G - 4 * t >= 4 else (G - 4 * t) * 32
                nc.scalar.activation(
                    out=t_h2T[0:nrows, t, :], in_=p2[0:nrows, :],
                    func=AF.Relu, bias=sb["ab2_8"][0:nrows, :],
                )

            # ---------------- attention mm3 (scores) ----------------
            t_scT = P.tile([8, NT, BC], f32, tag="scT")
            for t in range(NT):
                nl = min(8, L2 - 8 * t)  # 8, 8, 4
                p3 = PS.tile([8, BC], f32, tag="ps")
                nc.tensor.matmul(
                    out=p3[0:nl, :],
                    lhsT=sb["aw3p"][0 : (nl // 2) * 32, 0:nl],
                    rhs=t_h2T[0 : (nl // 2) * 32, t, :],
                    start=True, stop=True,
                )
                nc.scalar.copy(out=t_scT[0:nl, t, :], in_=p3[0:nl, :])

            # ---------- score transpose + softmax + pooling --------------
            t_w = P.tile([128, L2 * NB], f32, tag="w")  # col = l*NB + bb
            w_view = t_w[:].rearrange("p (l b) -> p b l", b=NB)
            for bb in range(NB):
                cols = slice(bb * 128, (bb + 1) * 128)
                psc = PS.tile([128, L2], f32, tag="psc")
                for t in range(NT):
                    nl = min(8, L2 - 8 * t)
                    nc.tensor.matmul(
                        out=psc[:, 8 * t : 8 * t + nl],
                        lhsT=t_scT[0:nl, t, cols],
                        rhs=t_ident[0:nl, 0:nl],
                        is_transpose=True,
                    )
                t_mask = W.tile([128, L2], mybir.dt.uint8, tag="mask")
                nc.vector.tensor_scalar(
                    out=t_mask[:], in0=sb["lidx"][:],
                    scalar1=sb["lenf"][:, bb : bb + 1], scalar2=None,
                    op0=ALU.is_lt,
                )
                t_sel = W.tile([128, L2], f32, tag="sel")
                nc.vector.select(
                    out=t_sel[:], mask=t_mask[:], on_true=psc[:],
                    on_false=t_negbig[:].to_broadcast([128, L2]),
                )
                t_nmax = W.tile([128, 1], f32, tag="nmax")
                nc.vector.tensor_reduce(
                    out=t_nmax[:], in_=t_sel[:], axis=mybir.AxisListType.X,
                    op=ALU.max, negate=True,
                )
                t_p = W.tile([128, L2], f32, tag="p")
                t_rs = W.tile([128, 1], f32, tag="rs")
                nc.scalar.activation(
                    out=t_p[:], in_=t_sel[:], func=AF.Exp,
                    bias=t_nmax[:], accum_out=t_rs[:],
                )
                t_winv = W.tile([128, 1], f32, tag="winv")
                nc.vector.reciprocal(out=t_winv[:], in_=t_rs[:])
                nc.vector.tensor_scalar(
                    out=w_view[:, bb, :], in0=t_p[:], scalar1=t_winv[:],
                    scalar2=None, op0=ALU.mult,
                )

                # pooling: tmp[l, d] = hist[l, d] * w[l]; reduce over l
                t_tmp = W.tile([128, L2, D], f32, tag="ptmp")
                h0 = t_hraw[:, bb * 2, :]  # chunk (l=0, bb): AP anchor
                hist_bb = _ap3(
                    h0, [h0.ap[0], [2 * NB * D, G], [D, 2], [1, D]]
                )
                w0 = t_w[:, bb : bb + 1]
                w_bb = _ap3(w0, [w0.ap[0], [2 * NB, G], [NB, 2], [0, D]])
                tmp0 = t_tmp[:, 0, :]
                tmp_o = _ap3(tmp0, [tmp0.ap[0], [2 * D, G], [D, 2], [1, D]])
                nc.vector.tensor_tensor(
                    out=tmp_o, in0=hist_bb, in1=w_bb, op=ALU.mult
                )
                t_pool = W.tile([128, D], f32, tag="pool")
                nc.vector.tensor_reduce(
                    out=t_pool[:],
                    in_=t_tmp[:].rearrange("p l d -> p d l"),
                    axis=mybir.AxisListType.X,
                    op=ALU.add,
                )
                pp = PT.tile([128, 128], f32, tag="pt")
                transpose(pp[0:64, :], t_pool[:])
                nc.scalar.copy(out=_mm(t_poolT[:, cols]), in_=pp[0:64, :])

            # ------- emb transposes + DNN chunks (paced by late gathers) --
            while emb_pairs:
                emit_emb_pair(*emb_pairs.pop(0))

            # ---------------- final DNN (chunks 0..10 already emitted) ----
            dnn_chunk(11, sb["dw1p"][0:64, 11, :], t_poolT[:], stop=True)
            t_x2 = P.tile([128, BC], f32, tag="x2")
            nc.scalar.activation(
                out=_mm(t_x2[:]), in_=pd1[:], func=AF.Relu, bias=sb["db1"][:]
            )
            pd2 = PB.tile([128, BC], f32, tag="pb")
            nc.tensor.matmul(
                out=pd2[0:64, :], lhsT=_mm(sb["dw2"][:]), rhs=_mm(t_x2[:]),
                start=True, stop=True,
            )
            t_x3 = P.tile([64, BC], f32, tag="x3")
            nc.scalar.activation(
                out=t_x3[:], in_=pd2[0:64, :], func=AF.Relu,
                bias=sb["db2"][:],
            )
            pd3 = PS.tile([1, BC], f32, tag="ps")
            nc.tensor.matmul(
                out=pd3[:], lhsT=sb["dw3"][:], rhs=t_x3[:],
                start=True, stop=True,
            )
            t_y = P.tile([1, BC], f32, tag="y")
            nc.vector.tensor_scalar(
                out=t_y[:], in0=pd3[:], scalar1=sb["db3"][0:1, :],
                scalar2=None, op0=ALU.add,
            )
            nc.sync.dma_start(out=y_dram[:], in_=t_y[:])

    nc.compile()
    return nc


# ---------------------------------------------------------------------------
# host-side prep
# ---------------------------------------------------------------------------

def sort_perms(inputs):
    """Per-core permutation sorting batch rows by hlen descending, plus the
    per-block max hlen plan (shared across cores: one SPMD program)."""
    perms = []
    maxlen = [0] * NB
    for c in range(NCORES):
        bsl = slice(c * BC, (c + 1) * BC)
        hlen = np.asarray(inputs["history_length"][bsl], dtype=np.int64)
        perm = np.argsort(-hlen, kind="stable")
        perms.append(perm)
        hs = hlen[perm]
        for bb in range(NB):
            maxlen[bb] = max(maxlen[bb], int(hs[bb * 128]))
    return perms, tuple(maxlen)


def make_core_inputs(inputs, c, perm, maxlen):
    hchunks = hchunks_for_plan(maxlen)
    bsl = slice(c * BC, (c + 1) * BC)
    sparse = np.asarray(inputs["sparse_inputs"][bsl], dtype=np.int64)[perm]
    hist = np.asarray(inputs["history"][bsl], dtype=np.int64)[perm]
    hlen = np.asarray(inputs["history_length"][bsl], dtype=np.int64)[perm]
    dense = np.asarray(inputs["dense_inputs"][bsl], dtype=np.float32)[perm]

    eidx = np.empty((128, EC), np.int32)
    for ci, (f, bb) in enumerate(ECHUNKS):
        eidx[:, ci] = f * V1 + sparse[bb * 128 : (bb + 1) * 128, f]
    hidx = np.empty((128, max(len(hchunks), 1)), np.int32)
    hidx[:, :] = 0
    for ci, (l, bb) in enumerate(hchunks):
        hidx[:, ci] = (l + 1) * V1 + hist[bb * 128 : (bb + 1) * 128, l]

    aw1 = np.asarray(inputs["aw1"], dtype=np.float32)
    aw2 = np.asarray(inputs["aw2"], dtype=np.float32)
    aw3 = np.asarray(inputs["aw3"], dtype=np.float32)
    ab1 = np.asarray(inputs["ab1"], dtype=np.float32)
    ab2 = np.asarray(inputs["ab2"], dtype=np.float32)
    dw1 = np.asarray(inputs["dw1"], dtype=np.float32)
    dw2 = np.asarray(inputs["dw2"], dtype=np.float32)
    dw3 = np.asarray(inputs["dw3"], dtype=np.float32)

    aw1q2 = np.concatenate([aw1[:D], aw1[:D]], axis=0)
    aw1h2 = np.concatenate([aw1[D:], aw1[D:]], axis=0)
    ab1_2 = np.concatenate([ab1, ab1])[:, None]
    aw2bd = np.zeros((128, 32), np.float32)
    aw2bd[0:64, 0:16] = aw2
    aw2bd[64:128, 16:32] = aw2
    ab2_8 = np.tile(ab2, 8)[:, None]
    aw3p = np.zeros((128, 8), np.float32)
    for q in range(4):
        for h in range(2):
            aw3p[q * 32 + h * 16 : q * 32 + h * 16 + 16, q * 2 + h] = aw3[:, 0]

    # dw1 row order per XT slots: slot0 = [emb f0 ; dense], k = [f2k-1 ; f2k],
    # slot 10 = f19 (top only), slot 11 = pooled (top only)
    dw1p = np.zeros((128, 12, 128), np.float32)
    dw1p[0:64, 0, :] = dw1[64:128]     # f0
    dw1p[64:128, 0, :] = dw1[0:64]     # dense
    for k in range(1, G):
        dw1p[:, k, :] = dw1[128 * k : 128 * (k + 1)]
    dw1p[0:64, 10, :] = dw1[1280:1344]  # f19
    dw1p[0:64, 11, :] = dw1[1344:1408]  # pooled

    lidx = np.broadcast_to(
        np.arange(L2, dtype=np.float32)[None, :], (128, L2)
    ).copy()
    lenf = np.ascontiguousarray(
        hlen.astype(np.float32).reshape(NB, 128).T
    )

    return {
        "table": inputs["_table_flat"],
        "eidx": eidx,
        "hidx": hidx,
        "denseT": np.ascontiguousarray(dense.T),
        "lidx": lidx,
        "lenf": lenf,
        "ident": np.eye(128, dtype=np.float32),
        "aw1q2": np.ascontiguousarray(aw1q2),
        "aw1h2": np.ascontiguousarray(aw1h2),
        "ab1_2": np.ascontiguousarray(ab1_2),
        "aw2bd": aw2bd,
        "ab2_8": np.ascontiguousarray(ab2_8),
        "aw3p": aw3p,
        "dw1p": dw1p,
        "db1": np.asarray(inputs["db1"], np.float32)[:, None],
        "dw2": dw2,
        "db2": np.asarray(inputs["db2"], np.float32)[:, None],
        "dw3": dw3,
        "db3": np.asarray(inputs["db3"], np.float32).reshape(1, 1),
    }


def prep_all_core_inputs(inputs):
    inputs = dict(inputs)
    inputs["_table_flat"] = np.ascontiguousarray(
        np.asarray(inputs["emb_tables"], dtype=np.float32).reshape(NF * V1, D)
    )
    perms, maxlen = sort_perms(inputs)
    return [
        make_core_inputs(inputs, c, perms[c], maxlen) for c in range(NCORES)
    ], perms, maxlen


_CACHED = {}


def kernel(**inputs) -> np.ndarray:
    maps, perms, maxlen = prep_all_core_inputs(inputs)
    nc = _CACHED.get(maxlen)
    if nc is None:
        nc = _CACHED[maxlen] = build_program(maxlen)
    res = run_bass_kernel_spmd(nc, maps, core_ids=list(range(NCORES)))
    out = np.empty(B, np.float32)
    for c in range(NCORES):
        y = res.results[c]["y"][0].astype(np.float32)
        inv = np.empty(BC, np.int64)
        inv[perms[c]] = np.arange(BC)
        out[c * BC : (c + 1) * BC] = y[inv]
    return out
